# revision 28
# baseline (speedup 1.0000x reference)
"""Trainium2 Bass kernel for nn_BandSplit (grouped band einsum as banded matmul).

The reference computes, per (b, t) row:
    g = gather(x, f_idxes) * mask            # per-band slice of the spectrum
    h = einsum('ki,kio->ko', g, pre_weight) + pre_bias
    y = einsum('ko,koj->kj', h, post_weight) + post_bias
    out = scatter_add(y * mask) / ola_window

Because each band's nonzero bins are a contiguous f-range, the whole pipeline
is linear in x and collapses to ONE banded matrix multiply in the interleaved
linear space  lin = f*4 + c  (bandwidth <= 131 < 132):

    out_lin[l', r] = sum_l A[l, l'] * x_lin[l, r]
    A = sum_k scatter(diag(mask_k) @ W1_k @ W2_k @ diag(mask_k / ola))

A is built on the host from the (small) weight inputs.  x is pre-transposed on
the host into [lin, rows] tiles so the device does only contiguous DMA plus
dense 128x128x512 fp32r matmuls on 3 block-diagonals (verified: no band
couples tiles further than +-1 apart).  Output tiles are disjoint across
cores.  The bias contribution is a per-(c, f) constant, added on the host.

Sharding: 2 row-halves x 4 lin-groups ([9,8,8,8] tiles of 128) over 8 cores.
Dtypes: x and weights ship as fp16 (SWDGE DMA upcasts to fp32r in-flight —
exact, since fp16's 10-bit mantissa fits fp32r's 11), PSUM accumulates fp32,
output ships fp16 (values are O(1); ~5e-4 relative error total).
"""

import numpy as np

# ---- problem constants (hardcoded; harness supplies matching inputs) ----
B, C, T, F = 4, 4, 512, 1025
KB, WMAX = 256, 33
L = F * C                 # 4100 linear positions
NT = (L + 127) // 128     # 33 tiles of 128
LPAD = NT * 128           # 4224
R = B * T                 # 2048 rows (b, t)
NCORES = 8
ND = 3                    # block diagonals
CHUNK = 512               # PSUM bank (fp32) free-dim limit

# The last lin-tile (32) covers only 4 real positions (f-bin 1024); its
# output is computed on the host, so the device grid is exactly 32 tiles.
NT_DEV = 32
RES_LO = NT_DEV * 128            # 4096: first host-residual out position
RES_IN0 = RES_LO - (WMAX - 1) * C - C + 1  # input support start (3965)


# grid: lin-groups x row-halves (set_grid recomputes the derived globals)
def set_grid(nling, nrowg):
    global NLING, NROWG, _TPG, _G0, NOUT, NIN, RC, NCHUNK, _prog_cache
    assert nling * nrowg == NCORES
    NLING, NROWG = nling, nrowg
    _TPG = [NT_DEV // nling + (1 if i < NT_DEV % nling else 0)
            for i in range(nling)]
    _G0 = [sum(_TPG[:i]) for i in range(nling)]
    NOUT = max(_TPG)
    NIN = NOUT + 2
    RC = R // nrowg
    NCHUNK = RC // CHUNK
    _prog_cache = {}


NLING = NROWG = _TPG = _G0 = NOUT = NIN = RC = NCHUNK = None
_prog_cache = {}
set_grid(8, 1)


# core id = rowg * NLING + ling
def _core_grid(cid):
    return cid // NLING, cid % NLING

# dtype plan
X_DT = "f16"     # "f32r" | "f16"
W_DT = "f16"     # "f32r" | "f16"
OUT_DT = "f16"   # "f32"  | "f16"
MM_DT = "f16"    # "f16": matmul directly in fp16 (no upcast, HWDGE loads);
                 # "f32r": upcast to fp32r during SWDGE DMA

_prog_cache = {}


def _build_program(loop_iters=1):
    """Uniform SPMD program: per core, NOUT out-tiles x 3 diagonals of
    [128,128] fp32r matmuls over [128,512] row chunks."""
    import concourse.bacc as bacc
    import concourse.tile as tile
    import concourse.mybir as mybir

    key = loop_iters
    if key in _prog_cache:
        return _prog_cache[key]

    f32 = mybir.dt.float32
    f32r = mybir.dt.float32r
    f16 = mybir.dt.float16

    x_dram_dt = f16 if X_DT == "f16" else f32r
    w_dram_dt = f16 if W_DT == "f16" else f32r
    out_dt = f16 if OUT_DT == "f16" else f32

    nc = bacc.Bacc("TRN2", target_bir_lowering=False, debug=False,
                   num_devices=NCORES)
    xin = nc.dram_tensor("xin", [NIN * 128, RC], x_dram_dt,
                         kind="ExternalInput").ap()
    wts = nc.dram_tensor("wts", [128, NOUT * ND * 128], w_dram_dt,
                         kind="ExternalInput").ap()
    out = nc.dram_tensor("out", [NOUT * 128, RC], out_dt,
                         kind="ExternalOutput").ap()

    with tile.TileContext(nc) as tc:
        with (
            tc.tile_pool(name="xp", bufs=1) as xp,
            tc.tile_pool(name="wp", bufs=1) as wp,
            tc.tile_pool(name="yp", bufs=3) as yp,
            tc.tile_pool(name="pp", bufs=8, space="PSUM") as pp,
        ):
            sbuf_mm_dt = f16 if MM_DT == "f16" else f32r

            def load(tile_ap, dram_slice):
                if MM_DT == "f16":
                    nc.sync.dma_start(tile_ap, dram_slice)   # raw fp16, HWDGE
                else:
                    nc.gpsimd.dma_start(tile_ap, dram_slice)  # SWDGE cast

            def body(_iv=None):
                # weights first: every matmul needs them
                wt = wp.tile([128, NOUT * ND * 128], sbuf_mm_dt, tag="w")
                load(wt[:], wts)
                xs = []
                for i in range(NIN):
                    t = xp.tile([128, RC], sbuf_mm_dt, tag=f"x{i}")
                    load(t[:], xin[i * 128:(i + 1) * 128, :])
                    xs.append(t)
                for j in range(NOUT):
                    y = yp.tile([128, RC], out_dt, tag="y")
                    for ch in range(NCHUNK):
                        ps = pp.tile([128, CHUNK], f32, tag="ps")
                        for d in range(ND):
                            blk = (j * ND + d) * 128
                            nc.tensor.matmul(
                                ps[:],
                                wt[:, blk:blk + 128],
                                xs[j + d][:, ch * CHUNK:(ch + 1) * CHUNK],
                                start=(d == 0), stop=(d == ND - 1),
                            )
                        dst = y[:, ch * CHUNK:(ch + 1) * CHUNK]
                        if (j * NCHUNK + ch) % 2 == 0:
                            nc.scalar.copy(dst, ps[:])
                        else:
                            nc.vector.tensor_copy(dst, ps[:])
                    nc.sync.dma_start(out[j * 128:(j + 1) * 128, :], y[:])

            if loop_iters == 1:
                body()
            else:
                with tc.For_i(0, loop_iters, 1) as _i:
                    body(_i)

    nc.compile()
    _prog_cache[key] = nc
    return nc


def _build_A(pre_weight, pre_bias, post_weight, post_bias, mask, ola_window,
             f_idxes):
    """Host: banded operator A[in_lin, out_lin] (LPAD x LPAD, fp32) and the
    constant bias image (C, F)."""
    fi = f_idxes.reshape(KB, WMAX).astype(np.int64)
    mk = mask.reshape(KB, WMAX).astype(np.float32)
    ola = ola_window.astype(np.float32)

    # effective per-band operators with mask and 1/ola folded in
    # row (input) index i = w*C + c ; col (output) index j = w'*C + c'
    mrow = np.repeat(mk, C, axis=1)                     # (KB, WMAX*C)
    inv_ola = np.where(ola != 0, 1.0 / ola, 0.0)
    ola_cols = inv_ola[fi]                              # (KB, WMAX)
    mcol = np.repeat(mk * ola_cols, C, axis=1)          # (KB, WMAX*C)

    w1 = pre_weight * mrow[:, :, None]                  # (KB, D, 128)
    w2 = post_weight * mcol[:, None, :]                 # (KB, 128, D)
    Mk = np.matmul(w1, w2)                              # (KB, D, D) fp32

    A = np.zeros((LPAD, LPAD), np.float32)
    lin = (fi[:, :, None] * C + np.arange(C)[None, None, :]).reshape(KB, -1)
    for k in range(KB):
        idx = lin[k]
        A[np.ix_(idx, idx)] += Mk[k]   # duplicate idx entries are all-zero rows/cols

    # bias: (pre_bias @ W2_raw + post_bias) * mask / ola, scattered -> (C, F)
    by = (np.einsum('ko,koj->kj', pre_bias, post_weight) + post_bias)  # (KB, D)
    by = by * mcol                                                      # masked + /ola
    bias_img = np.zeros((C, F), np.float32)
    np.add.at(bias_img,
              (np.tile(np.arange(C), (KB, WMAX, 1)).reshape(KB, -1),
               np.repeat(fi, C, axis=1)),
              by)
    return A, bias_img


def _round_fp32r(a):
    """Round fp32 to the fp32r format (11-bit mantissa, low 12 bits zero),
    round-to-nearest.  The PE reads only the top 20 bits; pre-rounding on the
    host keeps RNE accuracy instead of HW truncation."""
    b = np.ascontiguousarray(a, np.float32).view(np.uint32)
    r = (b + 0x7FF + ((b >> 12) & 1)) & np.uint32(0xFFFFF000)
    return r.view(np.float32)


def _shard_inputs(x, A):
    """Per-core xin ([NIN*128, RC]) and wts ([128, NOUT*ND*128]) arrays."""
    # x (B, C, T, F) -> X_lin [L, R], lin = f*4+c, r = b*T+t
    X = np.ascontiguousarray(
        x.transpose(3, 1, 0, 2).reshape(L, R).astype(np.float32))
    # rows: 128 front halo + LPAD + tail padding for the longest group window
    nrow_xp = (_G0[-1] + NIN + 1) * 128
    Xp = np.zeros((nrow_xp, R), np.float32)
    Xp[128:128 + L] = X                                   # halo offset 128
    Ap = np.zeros((LPAD + 256, LPAD), np.float32)
    Ap[128:128 + LPAD] = A

    # per lin-group weight blobs (shared by both row halves)
    wts_g = []
    for g in range(NLING):
        j0 = _G0[g]
        ntile = _TPG[g]
        wts = np.zeros((128, NOUT * ND * 128), np.float32)
        for j in range(ntile):
            gj = j0 + j
            for d in range(ND):
                blk = (j * ND + d) * 128
                wts[:, blk:blk + 128] = Ap[(gj + d) * 128:(gj + d + 1) * 128,
                                           gj * 128:(gj + 1) * 128]
        if W_DT == "f16":
            wts = wts.astype(np.float16)
        else:
            wts = _round_fp32r(wts)
        wts_g.append(wts)

    in_maps = []
    for cid in range(NCORES):
        rowg, ling = _core_grid(cid)
        j0 = _G0[ling]
        xsl = Xp[j0 * 128:(j0 + NIN) * 128, rowg * RC:(rowg + 1) * RC]
        if X_DT == "f16":
            xin_a = xsl.astype(np.float16)
        else:
            xin_a = _round_fp32r(xsl)
        in_maps.append({"xin": np.ascontiguousarray(xin_a),
                        "wts": wts_g[ling]})

    # host residual: the 4 real out positions of lin-tile 32 (f-bin 1024)
    residual = A[RES_IN0:L, RES_LO:L].T @ X[RES_IN0:L]    # [4, R] fp32
    return in_maps, residual


def _gather_output(results, bias_img, residual):
    out_lin = np.zeros((LPAD, R), np.float32)
    for cid in range(NCORES):
        rowg, ling = _core_grid(cid)
        j0, ntile = _G0[ling], _TPG[ling]
        out_lin[j0 * 128:(j0 + ntile) * 128, rowg * RC:(rowg + 1) * RC] = \
            results[cid]["out"][:ntile * 128].astype(np.float32)
    out_lin[RES_LO:L] = residual
    # [L, R] -> (B, C, T, F):  lin = f*4+c, r = b*T+t
    out = out_lin[:L].reshape(F, C, B, T).transpose(2, 1, 3, 0)
    out = np.ascontiguousarray(out) + bias_img[None, :, None, :]
    return out.astype(np.float32)


def _run_on_device(in_maps, loop_iters=1):
    from concourse.bass_utils import run_bass_kernel_spmd
    nc = _build_program(loop_iters)
    res = run_bass_kernel_spmd(nc, in_maps, list(range(NCORES)))
    return res.results


def kernel(x, pre_weight, pre_bias, post_weight, post_bias, mask, ola_window,
           f_idxes):
    x = np.asarray(x, np.float32)
    pre_weight = np.asarray(pre_weight, np.float32)
    pre_bias = np.asarray(pre_bias, np.float32)
    post_weight = np.asarray(post_weight, np.float32)
    post_bias = np.asarray(post_bias, np.float32)
    mask = np.asarray(mask, np.float32)
    ola_window = np.asarray(ola_window, np.float32)
    f_idxes = np.asarray(f_idxes)

    A, bias_img = _build_A(pre_weight, pre_bias, post_weight, post_bias,
                           mask, ola_window, f_idxes)
    in_maps, residual = _shard_inputs(x, A)
    results = _run_on_device(in_maps)
    return _gather_output(results, bias_img, residual)


# revision 29
# speedup vs baseline: 155.6868x; 155.6868x over previous
"""Trainium2 Bass kernel for nn_BandSplit (grouped band einsum as banded matmul).

The reference computes, per (b, t) row:
    g = gather(x, f_idxes) * mask            # per-band slice of the spectrum
    h = einsum('ki,kio->ko', g, pre_weight) + pre_bias
    y = einsum('ko,koj->kj', h, post_weight) + post_bias
    out = scatter_add(y * mask) / ola_window

Because each band's nonzero bins are a contiguous f-range, the whole pipeline
is linear in x and collapses to ONE banded matrix multiply in the interleaved
linear space  lin = f*4 + c  (bandwidth <= 131 < 132):

    out_lin[l', r] = sum_l A[l, l'] * x_lin[l, r]
    A = sum_k scatter(diag(mask_k) @ W1_k @ W2_k @ diag(mask_k / ola))

A is built on the host from the (small) weight inputs.  x is pre-transposed on
the host into [lin, rows] tiles so the device does only contiguous DMA plus
dense 128x128x512 fp32r matmuls on 3 block-diagonals (verified: no band
couples tiles further than +-1 apart).  Output tiles are disjoint across
cores.  The bias contribution is a per-(c, f) constant, added on the host.

Sharding: 2 row-halves x 4 lin-groups ([9,8,8,8] tiles of 128) over 8 cores.
Dtypes: x and weights ship as fp16 (SWDGE DMA upcasts to fp32r in-flight —
exact, since fp16's 10-bit mantissa fits fp32r's 11), PSUM accumulates fp32,
output ships fp16 (values are O(1); ~5e-4 relative error total).
"""

import numpy as np

# ---- problem constants (hardcoded; harness supplies matching inputs) ----
B, C, T, F = 4, 4, 512, 1025
KB, WMAX = 256, 33
L = F * C                 # 4100 linear positions
NT = (L + 127) // 128     # 33 tiles of 128
LPAD = NT * 128           # 4224
R = B * T                 # 2048 rows (b, t)
NCORES = 8
ND = 3                    # block diagonals
CHUNK = 512               # PSUM bank (fp32) free-dim limit

# The last lin-tile (32) covers only 4 real positions (f-bin 1024); its
# output is computed on the host, so the device grid is exactly 32 tiles.
NT_DEV = 32
RES_LO = NT_DEV * 128            # 4096: first host-residual out position
RES_IN0 = RES_LO - (WMAX - 1) * C - C + 1  # input support start (3965)


# grid: lin-groups x row-halves (set_grid recomputes the derived globals)
def set_grid(nling, nrowg):
    global NLING, NROWG, _TPG, _G0, NOUT, NIN, RC, NCHUNK, _prog_cache
    assert nling * nrowg == NCORES
    NLING, NROWG = nling, nrowg
    _TPG = [NT_DEV // nling + (1 if i < NT_DEV % nling else 0)
            for i in range(nling)]
    _G0 = [sum(_TPG[:i]) for i in range(nling)]
    NOUT = max(_TPG)
    NIN = NOUT + 2
    RC = R // nrowg
    NCHUNK = RC // CHUNK
    _prog_cache = {}


NLING = NROWG = _TPG = _G0 = NOUT = NIN = RC = NCHUNK = None
_prog_cache = {}
set_grid(8, 1)


# core id = rowg * NLING + ling
def _core_grid(cid):
    return cid // NLING, cid % NLING

# dtype plan
X_DT = "f16"     # "f32r" | "f16"
W_DT = "f16"     # "f32r" | "f16"
OUT_DT = "f16"   # "f32"  | "f16"
MM_DT = "f16"    # "f16": matmul directly in fp16 (no upcast, HWDGE loads);
                 # "f32r": upcast to fp32r during SWDGE DMA

_prog_cache = {}


def _build_program(loop_iters=1):
    """Uniform SPMD program: per core, NOUT out-tiles x 3 diagonals of
    [128,128] fp32r matmuls over [128,512] row chunks."""
    import concourse.bacc as bacc
    import concourse.tile as tile
    import concourse.mybir as mybir

    key = loop_iters
    if key in _prog_cache:
        return _prog_cache[key]

    f32 = mybir.dt.float32
    f32r = mybir.dt.float32r
    f16 = mybir.dt.float16

    x_dram_dt = f16 if X_DT == "f16" else f32r
    w_dram_dt = f16 if W_DT == "f16" else f32r
    out_dt = f16 if OUT_DT == "f16" else f32

    nc = bacc.Bacc("TRN2", target_bir_lowering=False, debug=False,
                   num_devices=NCORES)
    xin = nc.dram_tensor("xin", [NIN * 128, RC], x_dram_dt,
                         kind="ExternalInput").ap()
    wts = nc.dram_tensor("wts", [128, NOUT * ND * 128], w_dram_dt,
                         kind="ExternalInput").ap()
    out = nc.dram_tensor("out", [NOUT * 128, RC], out_dt,
                         kind="ExternalOutput").ap()

    with tile.TileContext(nc) as tc:
        with (
            tc.tile_pool(name="xp", bufs=1) as xp,
            tc.tile_pool(name="wp", bufs=1) as wp,
            tc.tile_pool(name="yp", bufs=3) as yp,
            tc.tile_pool(name="pp", bufs=8, space="PSUM") as pp,
        ):
            sbuf_mm_dt = f16 if MM_DT == "f16" else f32r

            def load(tile_ap, dram_slice):
                if MM_DT == "f16":
                    nc.sync.dma_start(tile_ap, dram_slice)   # raw fp16, HWDGE
                else:
                    nc.gpsimd.dma_start(tile_ap, dram_slice)  # SWDGE cast

            def body(_iv=None):
                # weights first: every matmul needs them
                wt = wp.tile([128, NOUT * ND * 128], sbuf_mm_dt, tag="w")
                load(wt[:], wts)
                xs = []
                for i in range(NIN):
                    t = xp.tile([128, RC], sbuf_mm_dt, tag=f"x{i}")
                    load(t[:], xin[i * 128:(i + 1) * 128, :])
                    xs.append(t)
                for j in range(NOUT):
                    y = yp.tile([128, RC], out_dt, tag="y")
                    for ch in range(NCHUNK):
                        ps = pp.tile([128, CHUNK], f32, tag="ps")
                        for d in range(ND):
                            blk = (j * ND + d) * 128
                            nc.tensor.matmul(
                                ps[:],
                                wt[:, blk:blk + 128],
                                xs[j + d][:, ch * CHUNK:(ch + 1) * CHUNK],
                                start=(d == 0), stop=(d == ND - 1),
                            )
                        dst = y[:, ch * CHUNK:(ch + 1) * CHUNK]
                        if (j * NCHUNK + ch) % 2 == 0:
                            nc.scalar.copy(dst, ps[:])
                        else:
                            nc.vector.tensor_copy(dst, ps[:])
                        # per-chunk store: overlaps the remaining chunks
                        nc.sync.dma_start(
                            out[j * 128:(j + 1) * 128,
                                ch * CHUNK:(ch + 1) * CHUNK], dst)

            if loop_iters == 1:
                body()
            else:
                with tc.For_i(0, loop_iters, 1) as _i:
                    body(_i)

    nc.compile()
    _prog_cache[key] = nc
    return nc


def _build_A(pre_weight, pre_bias, post_weight, post_bias, mask, ola_window,
             f_idxes):
    """Host: banded operator A[in_lin, out_lin] (LPAD x LPAD, fp32) and the
    constant bias image (C, F)."""
    fi = f_idxes.reshape(KB, WMAX).astype(np.int64)
    mk = mask.reshape(KB, WMAX).astype(np.float32)
    ola = ola_window.astype(np.float32)

    # effective per-band operators with mask and 1/ola folded in
    # row (input) index i = w*C + c ; col (output) index j = w'*C + c'
    mrow = np.repeat(mk, C, axis=1)                     # (KB, WMAX*C)
    inv_ola = np.where(ola != 0, 1.0 / ola, 0.0)
    ola_cols = inv_ola[fi]                              # (KB, WMAX)
    mcol = np.repeat(mk * ola_cols, C, axis=1)          # (KB, WMAX*C)

    w1 = pre_weight * mrow[:, :, None]                  # (KB, D, 128)
    w2 = post_weight * mcol[:, None, :]                 # (KB, 128, D)
    Mk = np.matmul(w1, w2)                              # (KB, D, D) fp32

    A = np.zeros((LPAD, LPAD), np.float32)
    lin = (fi[:, :, None] * C + np.arange(C)[None, None, :]).reshape(KB, -1)
    for k in range(KB):
        idx = lin[k]
        A[np.ix_(idx, idx)] += Mk[k]   # duplicate idx entries are all-zero rows/cols

    # bias: (pre_bias @ W2_raw + post_bias) * mask / ola, scattered -> (C, F)
    by = (np.einsum('ko,koj->kj', pre_bias, post_weight) + post_bias)  # (KB, D)
    by = by * mcol                                                      # masked + /ola
    bias_img = np.zeros((C, F), np.float32)
    np.add.at(bias_img,
              (np.tile(np.arange(C), (KB, WMAX, 1)).reshape(KB, -1),
               np.repeat(fi, C, axis=1)),
              by)
    return A, bias_img


def _round_fp32r(a):
    """Round fp32 to the fp32r format (11-bit mantissa, low 12 bits zero),
    round-to-nearest.  The PE reads only the top 20 bits; pre-rounding on the
    host keeps RNE accuracy instead of HW truncation."""
    b = np.ascontiguousarray(a, np.float32).view(np.uint32)
    r = (b + 0x7FF + ((b >> 12) & 1)) & np.uint32(0xFFFFF000)
    return r.view(np.float32)


def _shard_inputs(x, A):
    """Per-core xin ([NIN*128, RC]) and wts ([128, NOUT*ND*128]) arrays."""
    # x (B, C, T, F) -> X_lin [L, R], lin = f*4+c, r = b*T+t
    X = np.ascontiguousarray(
        x.transpose(3, 1, 0, 2).reshape(L, R).astype(np.float32))
    # rows: 128 front halo + LPAD + tail padding for the longest group window
    nrow_xp = (_G0[-1] + NIN + 1) * 128
    Xp = np.zeros((nrow_xp, R), np.float32)
    Xp[128:128 + L] = X                                   # halo offset 128
    Ap = np.zeros((LPAD + 256, LPAD), np.float32)
    Ap[128:128 + LPAD] = A

    # per lin-group weight blobs (shared by both row halves)
    wts_g = []
    for g in range(NLING):
        j0 = _G0[g]
        ntile = _TPG[g]
        wts = np.zeros((128, NOUT * ND * 128), np.float32)
        for j in range(ntile):
            gj = j0 + j
            for d in range(ND):
                blk = (j * ND + d) * 128
                wts[:, blk:blk + 128] = Ap[(gj + d) * 128:(gj + d + 1) * 128,
                                           gj * 128:(gj + 1) * 128]
        if W_DT == "f16":
            wts = wts.astype(np.float16)
        else:
            wts = _round_fp32r(wts)
        wts_g.append(wts)

    in_maps = []
    for cid in range(NCORES):
        rowg, ling = _core_grid(cid)
        j0 = _G0[ling]
        xsl = Xp[j0 * 128:(j0 + NIN) * 128, rowg * RC:(rowg + 1) * RC]
        if X_DT == "f16":
            xin_a = xsl.astype(np.float16)
        else:
            xin_a = _round_fp32r(xsl)
        in_maps.append({"xin": np.ascontiguousarray(xin_a),
                        "wts": wts_g[ling]})

    # host residual: the 4 real out positions of lin-tile 32 (f-bin 1024)
    residual = A[RES_IN0:L, RES_LO:L].T @ X[RES_IN0:L]    # [4, R] fp32
    return in_maps, residual


def _gather_output(results, bias_img, residual):
    out_lin = np.zeros((LPAD, R), np.float32)
    for cid in range(NCORES):
        rowg, ling = _core_grid(cid)
        j0, ntile = _G0[ling], _TPG[ling]
        out_lin[j0 * 128:(j0 + ntile) * 128, rowg * RC:(rowg + 1) * RC] = \
            results[cid]["out"][:ntile * 128].astype(np.float32)
    out_lin[RES_LO:L] = residual
    # [L, R] -> (B, C, T, F):  lin = f*4+c, r = b*T+t
    out = out_lin[:L].reshape(F, C, B, T).transpose(2, 1, 3, 0)
    out = np.ascontiguousarray(out) + bias_img[None, :, None, :]
    return out.astype(np.float32)


def _run_on_device(in_maps, loop_iters=1):
    from concourse.bass_utils import run_bass_kernel_spmd
    nc = _build_program(loop_iters)
    res = run_bass_kernel_spmd(nc, in_maps, list(range(NCORES)))
    return res.results


def kernel(x, pre_weight, pre_bias, post_weight, post_bias, mask, ola_window,
           f_idxes):
    x = np.asarray(x, np.float32)
    pre_weight = np.asarray(pre_weight, np.float32)
    pre_bias = np.asarray(pre_bias, np.float32)
    post_weight = np.asarray(post_weight, np.float32)
    post_bias = np.asarray(post_bias, np.float32)
    mask = np.asarray(mask, np.float32)
    ola_window = np.asarray(ola_window, np.float32)
    f_idxes = np.asarray(f_idxes)

    A, bias_img = _build_A(pre_weight, pre_bias, post_weight, post_bias,
                           mask, ola_window, f_idxes)
    in_maps, residual = _shard_inputs(x, A)
    results = _run_on_device(in_maps)
    return _gather_output(results, bias_img, residual)
